# revision 1
# baseline (speedup 1.0000x reference)
"""Trainium2 Bass kernel for nn_Action_Decoder (GAT-based action decoder).

Strategy (8 NeuronCores, pure data-parallel over batch):
  - B=4096 sharded 8 x 512 samples/core; all weights replicated.
  - One combined per-sample table [node(177) | sub(36)] in bf16; each tile
    does a single dma_gather(transpose=True) of 896 rows, landing directly
    in [channel, (k,b)] layout for the TensorEngine.
  - Layer-1 GAT: h^T = W1^T @ [obs_repr | sub | node] via PE matmuls;
    e_src/e_dst via PE matmuls against per-head selector matrices; h back
    to batch layout via PE transposes (hbL blocks in (j,h,f) order).
  - The per-sample 6x6x4-head softmax attention runs batch-on-partitions
    on the vector engine: per-destination products in hbL's native (j,h,f)
    layout (broadcast-alpha x contiguous-h runs at the DVE 1x rate), then
    a flat contiguous bf16 fold tree for the j-sum (2x packed mode).
  - elu (exact: exp(min(x,0)) + relu(x) - 1, the -1 folded into sum(W2))
    + Layer-2 GAT (128->1, single head) via affine_mul_reduce and a tiny
    batched 6x6 attention, split across vector/scalar engines.
"""

import os
import sys

import numpy as np

for _p in ("/root/.axon_site", "/root/.axon_site/_ro/trn_rl_repo",
           "/root/.axon_site/_ro/pypackages", "/opt/trn_rl_repo", "/opt/pypackages"):
    if os.path.isdir(_p) and _p not in sys.path:
        sys.path.append(_p)

import ml_dtypes

import concourse.bass as bass
import concourse.tile as tile
from concourse import bacc
from concourse import mybir
from concourse.bass_utils import run_bass_kernel_spmd

# Problem dims
B, N, S, K, H, OBS = 4096, 177, 36, 6, 128, 500
HEADS, FH = 4, 32
C_IN = 3 * H
NCORES = 8
BS = B // NCORES          # 512 samples per core
NT = BS // 128            # 4 tiles of 128 samples
OBS_PAD = 512             # pad 500 -> 512
R = N + S                 # combined table rows per sample (213)

F32 = mybir.dt.float32
BF16 = mybir.dt.bfloat16
I16 = mybir.dt.int16
AX = mybir.AxisListType
OP = mybir.AluOpType
ACT = mybir.ActivationFunctionType

LRELU_SLOPE = 0.2


def build_graph(scalars):
    as2 = float(scalars["a_src2"])
    ad2 = float(scalars["a_dst2"])
    b2 = float(scalars["b2"])
    c2 = float(scalars["c2"])

    nc = bacc.Bacc(num_swdge_queues=4)

    obs_T = nc.declare_dram_parameter("obs_T", [OBS_PAD, BS], BF16, isOutput=False)
    comb_emb = nc.declare_dram_parameter("comb_emb", [BS * R, H], BF16, isOutput=False)
    idx_comb = nc.declare_dram_parameter("idx_comb", [NT, 128, 56], I16, isOutput=False)
    w_proj = nc.declare_dram_parameter("w_proj", [OBS_PAD, H], BF16, isOutput=False)
    w1 = nc.declare_dram_parameter("w1", [C_IN, H], BF16, isOutput=False)
    asrc = nc.declare_dram_parameter("asrc", [H, 8], BF16, isOutput=False)
    bproj = nc.declare_dram_parameter("bproj", [H, 1], F32, isOutput=False)
    b1p = nc.declare_dram_parameter("b1p", [H, 1], F32, isOutput=False)
    ce = nc.declare_dram_parameter("ce", [1, 48], F32, isOutput=False)
    w2r = nc.declare_dram_parameter("w2r", [1, H], BF16, isOutput=False)
    ident_d = nc.declare_dram_parameter("ident_d", [128, 128], BF16, isOutput=False)
    out_ext = nc.declare_dram_parameter("out", [BS, K], F32, isOutput=True)

    with tile.TileContext(nc) as tc:
        from concourse import library_config
        nc.gpsimd.load_library(library_config.mlp)
        with (
            tc.tile_pool(name="consts", bufs=1) as consts,
            tc.tile_pool(name="obsp", bufs=1) as obsp,
            tc.tile_pool(name="gat", bufs=4) as gat,
            tc.tile_pool(name="work", bufs=4) as work,
            tc.tile_pool(name="big", bufs=2) as big,
            tc.tile_pool(name="small", bufs=3) as small,
            tc.tile_pool(name="psA", bufs=2, space="PSUM") as psA,
            tc.tile_pool(name="psB", bufs=2, space="PSUM") as psB,
            tc.tile_pool(name="psC", bufs=2, space="PSUM") as psC,
        ):
            # ---- idx tiles first (unblocks gathers), on scalar DGE ----
            idx_sb = consts.tile([128, NT, 56], I16)
            nc.scalar.dma_start(
                out=idx_sb[:, :, :],
                in_=bass.AP(tensor=idx_comb, offset=0,
                            ap=[[56, 128], [128 * 56, NT], [1, 56]]),
            )

            # ---- constants (batched DMAs) ----
            wproj_sb = consts.tile([128, 4, 128], BF16)
            nc.sync.dma_start(
                out=wproj_sb[:, :, :],
                in_=bass.AP(tensor=w_proj, offset=0,
                            ap=[[128, 128], [128 * 128, 4], [1, 128]]),
            )
            w1_sb = consts.tile([128, 3, 128], BF16)
            nc.sync.dma_start(
                out=w1_sb[:, :, :],
                in_=bass.AP(tensor=w1, offset=0,
                            ap=[[128, 128], [128 * 128, 3], [1, 128]]),
            )
            asrc_sb = consts.tile([128, 8], BF16)
            nc.sync.dma_start(out=asrc_sb, in_=asrc[:, :])
            bproj_sb = consts.tile([128, 1], F32)
            nc.sync.dma_start(out=bproj_sb, in_=bproj[:, :])
            b1_sb = consts.tile([128, 1], F32)
            nc.sync.dma_start(out=b1_sb, in_=b1p[:, :])
            ce_sb = consts.tile([128, 48], F32)
            nc.sync.dma_start(
                out=ce_sb,
                in_=bass.AP(tensor=ce, offset=0, ap=[[0, 128], [1, 48]]),
            )
            w2_sb = consts.tile([128, 128], BF16)
            nc.sync.dma_start(
                out=w2_sb,
                in_=bass.AP(tensor=w2r, offset=0, ap=[[0, 128], [1, 128]]),
            )
            ident = consts.tile([128, 128], BF16)
            nc.scalar.dma_start(out=ident, in_=ident_d[:, :])

            # ---- obs projection: obs_repr^T [H, BS] ----
            obs_in = obsp.tile([128, 4, BS], BF16)
            nc.sync.dma_start(
                out=obs_in[:, :, :],
                in_=bass.AP(tensor=obs_T, offset=0,
                            ap=[[BS, 128], [128 * BS, 4], [1, BS]]),
            )
            obs_ps = psA.tile([128, BS], F32, tag="hps")
            for c in range(4):
                nc.tensor.matmul(
                    obs_ps[:, :], wproj_sb[:, c, :], obs_in[:, c, :],
                    start=(c == 0), stop=(c == 3),
                )
            obs_sb = obsp.tile([128, BS], BF16)
            nc.scalar.activation(obs_sb[:, :], obs_ps[:, :], ACT.Identity,
                                 bias=bproj_sb[:, :], scale=1.0)

            # ---- per-tile pipeline ----
            for t in range(NT):
                # gathers: chunk A = node k0-3 + sub (640 cols), chunk B =
                # node k4-5 (256 cols) -- lets half-0 matmuls start early
                gth = gat.tile([128, 896], BF16)
                nc.gpsimd.dma_gather(
                    out_ap=gth[:, 0:640].rearrange("p (x n) -> p x n", x=1),
                    in_ap=comb_emb[t * 128 * R:(t + 1) * 128 * R, :],
                    idxs_ap=idx_sb[:, t, 0:40],
                    num_idxs=640, num_idxs_reg=640, elem_size=H,
                    transpose=True, queue_num=t,
                )
                nc.gpsimd.dma_gather(
                    out_ap=gth[:, 640:896].rearrange("p (x n) -> p x n", x=1),
                    in_ap=comb_emb[t * 128 * R:(t + 1) * 128 * R, :],
                    idxs_ap=idx_sb[:, t, 40:56],
                    num_idxs=256, num_idxs_reg=256, elem_size=H,
                    transpose=True, queue_num=t,
                )

                # h^T = W1^T @ x^T : PSUM [128 (h,f), 768 (k,b)]
                h_ps = psA.tile([128, 768], F32, tag="hps")
                for hs, nk in ((slice(0, 512), 4), (slice(512, 768), 2)):
                    rhs_obs = bass.AP(
                        tensor=obs_sb.tensor,
                        offset=obs_sb[:, :].offset + t * 128,
                        ap=[list(obs_sb[:, :].ap[0]), [0, nk], [1, 128]],
                    )
                    rhs_sub = bass.AP(
                        tensor=gth.tensor, offset=gth[:, :].offset + 512,
                        ap=[list(gth[:, :].ap[0]), [0, nk], [1, 128]],
                    )
                    nc.tensor.matmul(h_ps[:, hs], w1_sb[:, 0, :], rhs_obs,
                                     start=True, stop=False)
                    nc.tensor.matmul(h_ps[:, hs], w1_sb[:, 1, :], rhs_sub,
                                     start=False, stop=False)
                    node_rhs = (gth[:, 0:512] if hs.start == 0
                                else gth[:, 640:896])
                    nc.tensor.matmul(h_ps[:, hs], w1_sb[:, 2, :], node_rhs,
                                     start=False, stop=True)

                # h'^T (+b1) -> SBUF bf16
                hT_sb = work.tile([128, 768], BF16)
                nc.scalar.activation(hT_sb[:, :], h_ps[:, :], ACT.Identity,
                                     bias=b1_sb[:, :], scale=1.0)

                # e_src/e_dst per k-slice: PSUM [128b, 48=(k,sd,h)]
                e_ps = psB.tile([128, 48], F32)
                for k in range(K):
                    nc.tensor.matmul(
                        e_ps[:, k * 8:(k + 1) * 8],
                        hT_sb[:, k * 128:(k + 1) * 128], asrc_sb[:, :],
                        start=True, stop=True,
                    )
                e_sd = small.tile([128, 48], F32)
                nc.vector.tensor_sub(e_sd[:, :], e_ps[:, :], ce_sb[:, :])

                # h' to batch layout, strided so hbL is [128b, (h,f,j)]
                hbL_ps = psC.tile([128, 768], BF16)
                for k in range(K):
                    nc.tensor.transpose(
                        hbL_ps[:, k * 128:(k + 1) * 128],
                        hT_sb[:, k * 128:(k + 1) * 128], ident[:, :],
                    )
                hbL = work.tile([128, 768], BF16)
                nc.scalar.activation(hbL[:, :], hbL_ps[:, :], ACT.Copy)

                # ---- alpha phase (batch layout); E = exp(lrelu(e)) ----
                e_sd4 = e_sd[:, :].rearrange("p (k s h) -> p k s h", s=2, h=4)
                e_dst_ap = (
                    e_sd4[:, :, 1, :].unsqueeze(3).broadcast_to([128, 6, 4, 6])
                )
                e_src_ap = (
                    e_sd4[:, :, 0, :].unsqueeze(1)
                    .transpose([0, 1, 3, 2]).broadcast_to([128, 6, 4, 6])
                )
                e_raw = small.tile([128, 144], F32)
                nc.vector.tensor_tensor(
                    e_raw[:, :].rearrange("p (i h j) -> p i h j", h=4, j=6),
                    e_dst_ap, e_src_ap, OP.add,
                )
                e_lr = small.tile([128, 144], F32)
                nc.vector.scalar_tensor_tensor(
                    e_lr[:, :], e_raw[:, :], LRELU_SLOPE, e_raw[:, :],
                    OP.mult, OP.max)
                E1 = small.tile([128, 144], F32)
                nc.scalar.activation(E1[:, :], e_lr[:, :], ACT.Exp)
                E1v = E1[:, :].rearrange("p (ih j) -> p ih j", j=6)
                Z1 = small.tile([128, 24], F32)
                nc.vector.tensor_reduce(Z1[:, :], E1v, axis=AX.X, op=OP.add)
                rZ1 = small.tile([128, 24], F32)
                nc.vector.reciprocal_approx_fast(rZ1[:, :], Z1[:, :])
                al = small.tile([128, 144], BF16)
                nc.vector.tensor_mul(
                    al[:, :].rearrange("p (ih j) -> p ih j", j=6),
                    E1v,
                    rZ1[:, :].unsqueeze(2).broadcast_to([128, 24, 6]),
                )

                # ---- attention apply: products in hbL's native (j,h,f)
                # layout (contiguous in1 -> 1x rate), then a flat fold tree
                # for the j-sum (all-contiguous adds beat a strided reduce)
                alv = al[:, :].rearrange("p (i h j) -> p i h j", h=4, j=6)
                hv = hbL[:, :].rearrange("p (j h f) -> p j h f", h=4, f=32)
                attn = work.tile([128, 768], BF16)  # layout (i, h, f)
                for i in range(K):
                    prod = work.tile([128, 768], BF16, tag="prod")
                    nc.vector.tensor_mul(
                        prod[:, :].rearrange("p (j h f) -> p j h f",
                                             h=4, f=32),
                        alv[:, i, :, :].transpose([0, 2, 1]).unsqueeze(3)
                        .broadcast_to([128, 6, 4, 32]),
                        hv,
                    )
                    fs = work.tile([128, 384], BF16, tag="fs")
                    nc.vector.tensor_add(fs[:, :], prod[:, 0:384],
                                         prod[:, 384:768])
                    ft = work.tile([128, 128], BF16, tag="ft")
                    nc.vector.tensor_add(ft[:, :], fs[:, 0:128],
                                         fs[:, 128:256])
                    nc.vector.tensor_add(
                        attn[:, i * 128:(i + 1) * 128], ft[:, :],
                        fs[:, 256:384])

                # ---- elu(x) = exp(min(x,0)) + relu(x) - 1 (-1 folds into c2)
                relu_x = work.tile([128, 768], BF16)
                nc.scalar.activation(relu_x[:, :], attn[:, :], ACT.Relu)
                min_x = work.tile([128, 768], BF16)
                nc.vector.tensor_scalar_min(min_x[:, :], attn[:, :], 0.0)
                exp_m = work.tile([128, 768], BF16)
                nc.scalar.activation(exp_m[:, :], min_x[:, :], ACT.Exp)
                v1 = work.tile([128, 768], BF16)
                nc.vector.tensor_add(v1[:, :], exp_m[:, :], relu_x[:, :])

                # ---- layer 2: h2 = sum_hf (v-1)*W2 = amr(v,W2) - c2 ----
                h2 = small.tile([128, 6], F32)
                scr = work.tile([128, 128], F32)
                for i in range(K):
                    nc.vector.affine_mul_reduce(
                        out=scr[:, :], accum_out=h2[:, i:i + 1],
                        in0=v1[:, i * 128:(i + 1) * 128], in1=w2_sb[:, :],
                        scale=1.0, bias=0.0,
                    )
                h2c = small.tile([128, 6], F32)
                nc.vector.tensor_scalar(h2c[:, :], h2[:, :], -c2, None, OP.add)
                h2s = small.tile([128, 6], F32)
                nc.vector.tensor_scalar(h2s[:, :], h2[:, :], as2,
                                        -c2 * (as2 + ad2), OP.mult, OP.add)
                # e2 = ad2*h2[i] + (as2*h2[j] - c2*(as2+ad2))
                e2_raw = small.tile([128, 36], F32)
                nc.vector.scalar_tensor_tensor(
                    e2_raw[:, :].rearrange("p (i j) -> p i j", j=6),
                    h2[:, :].unsqueeze(2).broadcast_to([128, 6, 6]),
                    ad2,
                    h2s[:, :].unsqueeze(1).broadcast_to([128, 6, 6]),
                    OP.mult, OP.add,
                )
                e2_lr = small.tile([128, 36], F32)
                nc.vector.scalar_tensor_tensor(
                    e2_lr[:, :], e2_raw[:, :], LRELU_SLOPE, e2_raw[:, :],
                    OP.mult, OP.max)
                E2 = small.tile([128, 36], F32)
                nc.scalar.activation(E2[:, :], e2_lr[:, :], ACT.Exp)
                E2v = E2[:, :].rearrange("p (i j) -> p i j", j=6)
                Z2 = small.tile([128, 6], F32)
                nc.vector.tensor_reduce(Z2[:, :], E2v, axis=AX.X, op=OP.add)
                rZ2 = small.tile([128, 6], F32)
                nc.vector.reciprocal_approx_fast(rZ2[:, :], Z2[:, :])
                P2 = small.tile([128, 36], F32)
                nc.vector.tensor_mul(
                    P2[:, :].rearrange("p (i j) -> p i j", j=6),
                    E2v,
                    h2c[:, :].unsqueeze(1).broadcast_to([128, 6, 6]),
                )
                S2 = small.tile([128, 6], F32)
                nc.vector.tensor_reduce(
                    S2[:, :], P2[:, :].rearrange("p (i j) -> p i j", j=6),
                    axis=AX.X, op=OP.add,
                )
                out_sb = small.tile([128, 6], F32)
                nc.vector.tensor_mul(out_sb[:, :], S2[:, :], rZ2[:, :])
                if b2 != 0.0:
                    nc.vector.tensor_scalar(out_sb[:, :], out_sb[:, :], b2,
                                            None, OP.add)
                nc.sync.dma_start(out=out_ext[t * 128:(t + 1) * 128, :],
                                  in_=out_sb[:, :])

    nc.finalize()
    return nc


def prep_core_inputs(core, org_obs, node_embeddings, substation_embeddings,
                     sub_choice, sub_id_to_elem_id, W_proj, b_proj, W1,
                     a_src1, a_dst1, b1, W2, a_src2, a_dst2, b2):
    """Host-side shard + layout prep for one core (index math and weight
    folding only -- all tensor FLOPs stay on device)."""
    bf = ml_dtypes.bfloat16
    s = slice(core * BS, (core + 1) * BS)

    obs = org_obs[s]
    obs_T = np.zeros((OBS_PAD, BS), np.float32)
    obs_T[:OBS, :] = obs.T
    obs_T = obs_T.astype(bf)

    # combined per-sample table: [node(177) | sub(36)] rows
    comb = np.concatenate(
        [np.asarray(node_embeddings[s], np.float32),
         np.asarray(substation_embeddings[s], np.float32)], axis=1
    ).reshape(BS * R, H).astype(bf)

    sub_idx = sub_choice[s, 0].astype(np.int64)
    elem = sub_id_to_elem_id[sub_idx]              # [BS, K]

    idx_comb = np.zeros((NT, 128, 56), np.int16)
    for t in range(NT):
        bloc = np.arange(128)
        el = elem[t * 128:(t + 1) * 128]
        Ln = (bloc[None, :] * R + el.T)                        # [K, 128]
        Ls = bloc * R + N + sub_idx[t * 128:(t + 1) * 128]
        L = np.concatenate([Ln[:4].reshape(-1), Ls,
                            Ln[4:].reshape(-1)])               # 896
        blk = L.reshape(56, 16).T.astype(np.int16)
        idx_comb[t] = np.tile(blk, (8, 1))

    wp = np.zeros((OBS_PAD, H), np.float32)
    wp[:OBS] = W_proj
    return {
        "obs_T": obs_T,
        "comb_emb": comb,
        "idx_comb": idx_comb,
        "w_proj": wp.astype(bf),
        "w1": W1.astype(bf),
        "asrc": _asrc_mat(a_src1, a_dst1).astype(bf),
        "bproj": b_proj.reshape(H, 1).astype(np.float32),
        "b1p": b1.reshape(H, 1).astype(np.float32),
        "ce": _ce_const(b1, a_src1, a_dst1).reshape(1, 48).astype(np.float32),
        "w2r": np.asarray(W2, np.float32).reshape(1, H).astype(bf),
        "ident_d": np.eye(128, dtype=np.float32).astype(bf),
    }


def _asrc_mat(a_src1, a_dst1):
    m = np.zeros((H, 8), np.float32)
    for h in range(HEADS):
        m[h * FH:(h + 1) * FH, h] = a_src1[h]
        m[h * FH:(h + 1) * FH, 4 + h] = a_dst1[h]
    return m


def _ce_const(b1, a_src1, a_dst1):
    c = np.zeros((K, 2, HEADS), np.float32)
    b1r = np.asarray(b1, np.float32).reshape(HEADS, FH)
    c[:, 0, :] = (b1r * a_src1).sum(-1)[None, :]
    c[:, 1, :] = (b1r * a_dst1).sum(-1)[None, :]
    return c


_GRAPH_CACHE = {}
LAST_RESULTS = None


def kernel(**inputs):
    inp = {k: np.asarray(v) for k, v in inputs.items()}
    W2 = np.asarray(inp["W2"], np.float32)
    scalars = {
        "a_src2": float(np.asarray(inp["a_src2"]).reshape(-1)[0]),
        "a_dst2": float(np.asarray(inp["a_dst2"]).reshape(-1)[0]),
        "b2": float(np.asarray(inp["b2"]).reshape(-1)[0]),
        "c2": float(W2.sum()),
    }
    key = tuple(sorted(scalars.items()))
    if key not in _GRAPH_CACHE:
        _GRAPH_CACHE[key] = build_graph(scalars)
    nc = _GRAPH_CACHE[key]

    in_maps = [
        prep_core_inputs(
            c, inp["org_obs"], inp["node_embeddings"],
            inp["substation_embeddings"], inp["sub_choice"],
            inp["sub_id_to_elem_id"], inp["W_proj"], inp["b_proj"], inp["W1"],
            inp["a_src1"], inp["a_dst1"], inp["b1"], inp["W2"], inp["a_src2"],
            inp["a_dst2"], inp["b2"],
        )
        for c in range(NCORES)
    ]
    res = run_bass_kernel_spmd(nc, in_maps, core_ids=list(range(NCORES)))
    global LAST_RESULTS
    LAST_RESULTS = res
    out = np.concatenate([res.results[c]["out"] for c in range(NCORES)], axis=0)
    return out.reshape(B, K, 1).astype(np.float32)


if __name__ == "__main__":
    g = build_graph({"a_src2": 0.01, "a_dst2": 0.02, "b2": 0.0, "c2": 0.1})
    print("graph built ok")

